# revision 44
# baseline (speedup 1.0000x reference)
"""Trainium2 Bass kernel (v6) for nn_Attn: batched column-softmax attention energies.

Math (per batch element b):
    E = encoder_outputs[:, b, :]            # [H, T]
    d = decoder_hidden[b]                   # [H]
    s = E^T d                               # [T]  (scores)
    w[h, t] = E[h, t] * s[t]
    sm = softmax over h of w (per column t)
    out[b, h] = sum_t sm[h, t]

Design (per core, data parallel over batch: 8 cores x 8 batch elements):
    - v6: E is cast to float16 on the HOST in make_in_maps, so DRAM holds
      16MB/core instead of 32MB -- HBM read traffic (the real-HW binding
      resource under device contention) is HALVED, and the loads become
      plain HWDGE (sync) DMAs with 2KB-contiguous descriptors instead of
      SWDGE casting DMAs (~2us fixed cost each, Q7 descriptor generation).
      Numerically identical to the old device-side casting DMA (both RNE):
      HW rel err 1.0e-2 vs the 2e-2 gate (bf16 E fails at 3.8e-2).
    - E transposes as fp16 transpose-mode matmuls: 1.0 cyc/row (vs 2.0 fp32),
      each Et PSUM tile one bank (pp_bufs=3 deepens the pipeline).
    - scores on PE: sneg_row[1,T] = sum_i (-d_i)^T @ E_i per T-half, then tiny
      fp32 PE transposes give per-partition score columns scT[128, NTC].
      (score_lazy / nat_bufs=5 / last-b lag=1 were tried and are model-neutral;
      left OFF -- only model-positive, HW-A/B-validated changes ship.)
    - DVE: ONE fused pass per chunk: junk(fp16) = -s*Et, accum min -> mneg =
      -max_h(s*Et); fp16 PSUM in + fp16 out -> 2x_1p mode. Plus recip 1/z,
      bf16 cast, and s-score PSUM->SBUF copies.
    - ACT: e = exp(-junk + mneg) as bf16, accum z (z >= 1). v6: ACT does
      ONLY the 64 exps -- the old [1,512] output-row copies are gone.
    - v6 output path (o_cols): final matmuls are FLIPPED -- lhsT = e-block
      [128t,128h] (stationary), rhs = rl [128,1] -> out COLUMN [128,1] per
      h-block, accumulated over t-chunks into one PSUM bank [128, NHC].
      TRN2 allows one accumulation group per 2KB zero region, so the group
      is opened once per b by a full-region zero write (I^T @ 0, start=True);
      all 64 column matmuls join it (start=False) and the last one stops.
      Flush per b = tiny DVE copy [128,8] + one small PE transpose ->
      [8,128] + DVE copy + a DMA with 8x512B descriptors. This removes
      ~10us/core of ACT row-copies vs v5 (ACT busy 88.2 -> 78.4us) at the
      cost of ~2us of DVE/PE dust; flush is deferred into the next b's
      chunk loop (o_defer) so it never blocks the softmax-critical DVE chain.
    - back-half ops (recip/cast/final matmuls) are emitted 2 chunks late so
      the in-order PE/DVE queues never stall on the softmax chain.
    - pe_warm dummy transposes bridge the decT->scores PE idle gap (~3.3us,
      right at the HAM 3.4us re-throttle window); score_head_split emits a
      chunk-0-only b0 scores piece so the first exp fires ~1.5us earlier.
    - engine busy (TimelineSim cost model, per core): ACT 78.4us (critical,
      zero mid-gaps; 64 exps at (1024+446)cyc -- the fixed part is intrinsic
      instruction overhead), PE ~57us modeled (~82us real incl unmodeled
      LDWEIGHTS), DVE 58us, DMA 47us; predicted total 93.0us vs 105.9us for
      v5. Remaining slack is the DMA-gated head (~10us) and the flush tail
      (~4us). Real-HW marginal timing is dominated by shared-device noise
      (210us..3.5ms for the SAME kernel minutes apart); best observed v6:
      210us vs v5 baseline 333us.
"""

import numpy as np

import concourse.bass as bass
import concourse.mybir as mybir
from concourse.bass_utils import run_bass_kernel_spmd
from concourse.tile import TileContext

H = 1024
B = 64
T = 1024
N_CORES = 8
B_LOC = B // N_CORES  # 8 batch elements per core
NHC = H // 128        # 8 h-chunks
NTC = T // 128        # 8 t-chunks

F32 = mybir.dt.float32
F32R = mybir.dt.float32r
F16 = mybir.dt.float16
BF16 = mybir.dt.bfloat16


def _split_waits(nc, max_waits=1):
    """Workaround for this container's walrus: instructions accept only one
    sync-wait; hoist extra waits onto single-wait Drain carriers."""
    n_new = 0
    for f in nc.m.functions:
        for blk in f.blocks:
            new_insts = []
            for inst in blk.instructions:
                si = inst.sync_info
                if si is not None and si.on_wait is not None and len(si.on_wait) > max_waits:
                    waits = list(si.on_wait)
                    while len(waits) > max_waits:
                        w = waits.pop(0)
                        d = mybir.InstDrain(
                            name=f"I-ws-{nc.next_id()}", ins=[], outs=[]
                        )
                        d.engine = inst.engine
                        d.sync_info = mybir.SyncInfo(on_wait=[w], on_update=[])
                        new_insts.append(d)
                        n_new += 1
                    si.on_wait = waits
                new_insts.append(inst)
            blk.instructions = new_insts
    return n_new


def build_program(
    host_cast=True,      # enc pre-cast to fp16 on HOST: halves HBM traffic, HWDGE loads
    f32r=True,           # f32r natt (casting DMA) + f32r transposes + f32r s-matmuls
    f16=True,            # fp16 natt (1 cyc/row transposes, 1-bank Et PSUM)
    score_pe=True,       # scores via PE (else DVE pass like v1)
    o_compact=True,      # out accum as [2,512] in one PSUM bank, double buffered
    split_nat_dma=2,     # natt loaded in this many DMAs (pipeline head start; 2 aligns with the score-row halves and halves SWDGE descriptor count vs 4)
    rl_engine="vector",
    lag=2,               # chunks of emission lag for recip/cast/final-matmul
    nat_bufs=3,
    pp_bufs=3,
    w16=True,            # fp16 junk (-s*Et scratch): 2-byte in+out => DVE 2x_1p mode
    s2_engine="vector",   # engine for s2 PSUM->SBUF copies: scalar|vector
    o_direct_dma=False,   # dead: bass forbids PSUM-source DMA (in_ must be SBUF/DRAM)
    orow_engine="scalar", # engine for o PSUM->SBUF copies: scalar|vector (vector mis-executes the partition-32 copy on real HW)|vector33 (ONE DVE copy spanning partitions 0..32, base 0)|scalar33
    nat_prefetch=False,  # issue b=0 natt DMAs before ident/dec (head start)
    o_defer=True,        # emit b's orow copy + out DMA inside b+1's chunk loop
    o_cols=True,         # out accum as PSUM COLUMNS [128, NHC] (lhsT=e-block,
                         # rhs=rl): flush = tiny DVE copy + small PE transpose
                         # + 8x512B DMA; removes the [1,512] ACT row copies
    score_lazy=False,    # b=0: emit scores piece 1 after chunk-2's exp
                         # (model-neutral; measured noisy-negative on HW -> off)
    pe_warm=18,          # dummy identsb transposes after decT so PE spans the
                         # HAM warmup window busy and the b0 scores run at 2.4GHz
    score_head_split=True,  # b=0: emit a chunk-0-only scores piece first
    b0_sliver=True,      # b=0 loads as 256+256+512 so chunk-0 waits on 0.5MB
    head_quarters=False, # b=0 quarter-granular head: helps pre-split-waits (-1.2us) but the wait-split drains land worse (+0.8us net) -> off
    junk_bufs=3,
    e_bufs=4,
    small_bufs=8,
    split_waits=True,
):
    nc = bass.Bass("TRN2", debug=False, num_devices=N_CORES)
    if host_cast:
        f16 = True
    enc_h = nc.dram_tensor(
        "enc", [H, B_LOC, T], F16 if host_cast else F32, kind="ExternalInput"
    )
    dec_h = nc.dram_tensor("dec", [B_LOC, H], F32, kind="ExternalInput")
    ident_h = nc.dram_tensor("ident", [128, 128], F32, kind="ExternalInput")
    out_h = nc.dram_tensor("out", [B_LOC, H], F32, kind="ExternalOutput")

    enc = enc_h.ap()
    dec = dec_h.ap()
    ident = ident_h.ap()
    out = out_h.ap()

    AF = mybir.ActivationFunctionType
    OP = mybir.AluOpType

    if f16:
        f32r = False
    NAT_DT = F16 if f16 else (F32R if f32r else F32)
    PP_DT = F16 if f16 else F32

    def mm_trans(out_ap, lhsT, rhs, **kw):
        if f32r:
            out_ap = out_ap.bitcast(F32R)
        nc.tensor.matmul(out_ap, lhsT=lhsT, rhs=rhs, is_transpose=True, **kw)

    with TileContext(nc) as tc:
        with (
            tc.tile_pool(name="const", bufs=1) as constp,
            tc.tile_pool(name="natp", bufs=nat_bufs) as natp,
            tc.tile_pool(name="junkp", bufs=junk_bufs) as junkp,
            tc.tile_pool(name="ep", bufs=e_bufs) as ep,
            tc.tile_pool(name="srowp", bufs=2) as srowp,
            tc.tile_pool(name="sctp", bufs=2) as sctp,
            tc.tile_pool(name="smallp", bufs=small_bufs) as smallp,
            tc.tile_pool(name="rowp", bufs=2) as rowp,
            tc.tile_pool(name="dbp", bufs=2) as dbp,  # only if not score_pe
            tc.tile_pool(name="ps_p", bufs=pp_bufs, space="PSUM") as ps_p,
            tc.tile_pool(name="ps_o", bufs=2 if (o_compact or o_cols) else 1, space="PSUM") as ps_o,
            tc.tile_pool(name="ps_s", bufs=2, space="PSUM") as ps_s,   # 2 banks
            tc.tile_pool(name="ps_x", bufs=1, space="PSUM") as ps_x,   # out transpose
        ):
            # issue b=0's natt loads FIRST so the big transfer heads the HWDGE
            # queue (ident/dec are tiny and their consumers run later anyway)
            natt_pre = None
            if host_cast and nat_prefetch:
                # head order: natt[b0] first half -> ident+dec (tiny) -> rest,
                # so scores piece 0 and decT are both ready ~as early as possible
                enc_b0 = enc[:, 0, :].rearrange("(ii p) t -> p ii t", p=128)
                natt_pre = natp.tile([128, NHC, T], NAT_DT, name="natt", tag="nat")
                tw0 = T // split_nat_dma
                nc.sync.dma_start(out=natt_pre[:, :, 0:tw0], in_=enc_b0[:, :, 0:tw0])

            # ident/dec stay on the sync ring by default; under nat_prefetch
            # they ride the idle SWDGE (gpsimd) dispatcher so the natt[b0]
            # transfer heads the sync HWDGE ring alone (ACT ring is unusable:
            # DMA dispatches there pollute the exp-critical ACT queue)
            aux_dma = nc.gpsimd if nat_prefetch else nc.sync
            identsb = constp.tile([128, 128], F32, name="identsb")
            aux_dma.dma_start(out=identsb[:, :], in_=ident)
            zero_sb = None
            if o_cols:
                zero_sb = constp.tile([128, NHC], F32, name="zero_sb")
                nc.vector.tensor_scalar(zero_sb[:, :], identsb[:, 0:NHC], 0.0, None, OP.mult)
            if f32r or f16:
                identr = constp.tile([128, 128], NAT_DT, name="identr")
                nc.vector.tensor_scalar(identr[:, :], identsb[:, :], 1.0, None, OP.mult)
            else:
                identr = identsb

            decTn_sb = None
            if score_pe:
                # dec natural [B_LOC, H] (one contiguous 32KB DMA)
                dec_nat = constp.tile([B_LOC, H], F32, name="dec_nat")
                aux_dma.dma_start(out=dec_nat[:, :], in_=dec)
                # decT[p, i, b] = d[b, 128i+p]; negate while copying to SBUF
                decT_ps = ps_s.tile([128, NHC, B_LOC], F32, name="decT_ps", tag="ps_s")
                for i in range(NHC):
                    nc.tensor.matmul(
                        decT_ps[:, i, :],
                        lhsT=dec_nat[:, 128 * i : 128 * (i + 1)],
                        rhs=identsb[0:B_LOC, 0:B_LOC],
                        is_transpose=True,
                    )
                decTn_sb = constp.tile([128, NHC, B_LOC], NAT_DT, name="decTn_sb")
                nc.vector.tensor_scalar(
                    decTn_sb[:, :, :], decT_ps[:, :, :], -1.0, None, OP.mult
                )

            if natt_pre is not None:
                tw0 = T // split_nat_dma
                enc_b0 = enc[:, 0, :].rearrange("(ii p) t -> p ii t", p=128)
                for q in range(1, split_nat_dma):
                    tsl = slice(q * tw0, (q + 1) * tw0)
                    nc.sync.dma_start(out=natt_pre[:, :, tsl], in_=enc_b0[:, :, tsl])

            if pe_warm:
                # keep the PE array busy from decT until natt[b0] lands so the
                # HAM clock gate stays open (cold matmuls run at 1.2 not 2.4GHz)
                warm_ps = ps_x.tile([128, 128], F32, name="warm_ps", tag="ps_x")
                for _ in range(pe_warm):
                    nc.tensor.matmul(
                        warm_ps[:, :], lhsT=identsb[:, :], rhs=identsb[:, :],
                        is_transpose=True,
                    )

            def flush_out(bprev, o_psprev):
                if o_cols:
                    # o_psprev [128, NHC]: col i holds out[b, 128i + p] at
                    # partition p. Copy out (tiny), transpose to [NHC, 128]
                    # so the DMA writes 8 contiguous 512B runs.
                    o_sb = rowp.tile([128, NHC], F32, name="o_sb", tag="orow")
                    nc.vector.tensor_scalar(o_sb[:, :], o_psprev[:, :], 1.0, None, OP.mult)
                    x_ps = ps_x.tile([NHC, 128], F32, name="x_ps", tag="ps_x")
                    nc.tensor.matmul(
                        x_ps[:, :], lhsT=o_sb[:, :], rhs=identsb[:, :],
                        is_transpose=True,
                    )
                    x_sb = rowp.tile([NHC, 128], F32, name="x_sb", tag="orow")
                    nc.vector.tensor_scalar(x_sb[:, :], x_ps[:, :], 1.0, None, OP.mult)
                    out_b = out[bprev : bprev + 1, :].rearrange(
                        "o (ii p) -> (o ii) p", p=128
                    )
                    nc.sync.dma_start(out=out_b, in_=x_sb[:, :])
                    return
                if o_compact and o_direct_dma:
                    # straight PSUM -> DRAM, no SBUF staging
                    nc.sync.dma_start(out=out[bprev : bprev + 1, 0:512], in_=o_psprev[0:1, :])
                    nc.sync.dma_start(out=out[bprev : bprev + 1, 512:1024], in_=o_psprev[32:33, :])
                elif o_compact:
                    orow = rowp.tile([33, 512], F32, name="orow", tag="orow")
                    if orow_engine in ("strided", "vstrided"):
                        # one copy for both output rows via a partition-strided
                        # AP (base partition 0, stride 32) into rows {0,32}
                        if orow_engine == "strided":
                            nc.scalar.copy(orow[0:33:32, :], o_psprev[0:33:32, :])
                        else:
                            nc.vector.tensor_scalar(
                                orow[0:33:32, :], o_psprev[0:33:32, :], 1.0, None, OP.mult
                            )
                    elif orow_engine == "mixed":
                        # partition-0 row on DVE (safe base-0 AP), row 32 on ACT
                        nc.vector.tensor_scalar(orow[0:1, :], o_psprev[0:1, :], 1.0, None, OP.mult)
                        nc.scalar.copy(orow[32:33, :], o_psprev[32:33, :])
                    elif orow_engine == "vector33":
                        nc.vector.tensor_scalar(orow[0:33, :], o_psprev[0:33, :], 1.0, None, OP.mult)
                    elif orow_engine == "scalar33":
                        nc.scalar.copy(orow[0:33, :], o_psprev[0:33, :])
                    elif orow_engine == "vector":
                        nc.vector.tensor_scalar(orow[0:1, :], o_psprev[0:1, :], 1.0, None, OP.mult)
                        nc.vector.tensor_scalar(orow[32:33, :], o_psprev[32:33, :], 1.0, None, OP.mult)
                    else:
                        nc.scalar.copy(orow[0:1, :], o_psprev[0:1, :])
                        nc.scalar.copy(orow[32:33, :], o_psprev[32:33, :])
                    out_b = out[bprev : bprev + 1, :].rearrange("o (p f) -> (o p) f", p=2)
                    nc.sync.dma_start(out=out_b, in_=orow[0:33:32, :])
                else:
                    orow = rowp.tile([1, H], F32, name="orow", tag="orow")
                    nc.scalar.copy(orow[:, :], o_psprev[0:1, :])
                    nc.sync.dma_start(out=out[bprev : bprev + 1, :], in_=orow[:, :])

            pending_flush = None
            for b in range(B_LOC):
                # ---- natural-layout load: natt[p, i, t] = E[128*i + p, t]
                # (f32r: gpsimd SWDGE casting DMA rounds fp32 -> f32r in flight)
                enc_b = enc[:, b, :].rearrange("(ii p) t -> p ii t", p=128)
                if b == 0 and natt_pre is not None:
                    natt = natt_pre
                else:
                    natt = natp.tile([128, NHC, T], NAT_DT, name="natt", tag="nat")
                    if b == 0 and b0_sliver and host_cast:
                        # first load sliced [0:256]+[256:512]+[512:1024]: the
                        # chunk-0 score/transpose chain only waits on the first
                        # 0.5MB (512B descriptors -- still at line-rate size)
                        slices = [(0, 256), (256, 256), (512, 512)]
                    else:
                        nsplit = 4 if (head_quarters and b == 0) else split_nat_dma
                        tw = T // nsplit
                        slices = [(q * tw, tw) for q in range(nsplit)]
                    for lo, w in slices:
                        tsl = slice(lo, lo + w)
                        if host_cast:
                            # enc already fp16 in DRAM: plain HWDGE load, no cast
                            nc.sync.dma_start(out=natt[:, :, tsl], in_=enc_b[:, :, tsl])
                        elif f32r or f16:
                            nc.gpsimd.dma_start(out=natt[:, :, tsl], in_=enc_b[:, :, tsl])
                        else:
                            nc.sync.dma_start(out=natt[:, :, tsl], in_=enc_b[:, :, tsl])

                emit_piece = None
                lazy_pieces = {}
                if score_pe:
                    # ---- sneg_row[1, T] = sum_i (-d_i)^T @ E_i, per T-half so
                    # chunks j<4 only wait on natt's first half (pipeline head)
                    # (kept fp32: the [1,128]->[128,1] transposes below violate
                    # walrus's fp32r ISA restrictions in f32r form)
                    srow_sb = srowp.tile([1, T], F32, name="srow_sb", tag="srow")
                    scTn_sb = sctp.tile([128, NTC], F32, name="scTn_sb", tag="sct")
                    def emit_piece(lo, pw, natt=natt, srow_sb=srow_sb,
                                   scTn_sb=scTn_sb, b=b):
                        jlo, jpp = lo // 128, pw // 128
                        tsl = slice(lo, lo + pw)
                        s2 = ps_s.tile([1, pw], F32, name="s2", tag="ps_s")
                        for i in range(NHC):
                            nc.tensor.matmul(
                                s2[0:1, :],
                                lhsT=decTn_sb[:, i, b : b + 1],
                                rhs=natt[:, i, tsl],
                                start=(i == 0),
                                stop=(i == NHC - 1),
                            )
                        if s2_engine == "vector":
                            nc.vector.tensor_scalar(srow_sb[0:1, tsl], s2[0:1, :], 1.0, None, OP.mult)
                        else:
                            nc.scalar.copy(srow_sb[0:1, tsl], s2[0:1, :])
                        # transpose this piece's scores to columns:
                        # scT[t_p, j] = sneg[128j + t_p]
                        scT_ps = ps_s.tile([128, jpp], F32, name="scT_ps", tag="ps_s")
                        for jj in range(jpp):
                            j = jlo + jj
                            nc.tensor.matmul(
                                scT_ps[:, jj : jj + 1],
                                lhsT=srow_sb[0:1, 128 * j : 128 * (j + 1)],
                                rhs=identsb[0:1, 0:1],
                                is_transpose=True,
                            )
                        nc.vector.tensor_scalar(
                            scTn_sb[:, jlo : jlo + jpp],
                            scT_ps[:, :], 1.0, None, OP.mult,
                        )

                    if score_head_split and b == 0:
                        # chunk-0 scores alone first: the exp-critical path is
                        # 8 narrow matmuls instead of a 512-wide piece
                        pieces = [(0, 128), (128, 384), (512, 512)]
                    else:
                        pieces = [(0, 512), (512, 512)]
                    for idx, (lo, pw) in enumerate(pieces):
                        if score_lazy and b == 0 and lo >= 512:
                            lazy_pieces[lo // 128 - 2] = (lo, pw)
                        else:
                            emit_piece(lo, pw)
                else:
                    dbcast = dbp.tile([128, H], F32, name="dbcast", tag="dbcast")
                    nc.sync.dma_start(
                        out=dbcast[:, :], in_=dec[b : b + 1, :].to_broadcast([128, H])
                    )

                # ---- per t-chunk: transpose, softmax, accumulate output
                if o_cols:
                    o_ps = ps_o.tile([128, NHC], F32, name="o_ps", tag="ps_o")
                    # one accumulation group for the whole [128, NHC] zero
                    # region: open it with a full-region zero write (I.T @ 0)
                    nc.tensor.matmul(
                        o_ps[:, :], lhsT=identsb[:, :], rhs=zero_sb[:, :],
                        start=True, stop=False,
                    )
                elif o_compact:
                    o_ps = ps_o.tile([128, 512], F32, name="o_ps", tag="ps_o")
                else:
                    o_ps = ps_o.tile([1, H], F32, name="o_ps", tag="ps_o")
                # Per chunk j: [transposes (PE), fused mult+min (DVE), exp (ACT)]
                # emitted at step j; [recip, bf16 cast (DVE), 2 final matmuls
                # (PE)] emitted `lag` chunks later so the in-order PE/DVE
                # queues never stall waiting for the softmax chain of the
                # current chunk.
                pend = {}
                b_lag = lag
                for step in range(NTC + b_lag):
                    if step == 1 and pending_flush is not None:
                        flush_out(*pending_flush)
                        pending_flush = None
                    if step < NTC:
                        j = step
                        p_ps = ps_p.tile([128, H], PP_DT, name="p_ps", tag="ps_p")
                        for i in range(NHC):
                            mm_trans(
                                p_ps[:, 128 * i : 128 * (i + 1)],
                                lhsT=natt[:, i, 128 * j : 128 * (j + 1)],
                                rhs=identr[:, :],
                                start=(i % 4 == 0),
                                stop=(i % 4 == 3),
                            )
                        junk = junkp.tile([128, H], F16 if (w16 and f16) else F32,
                                          name="junk", tag="junk")
                        mneg = smallp.tile([128, 1], F32, name="mneg", tag="mneg")
                        if score_pe:
                            nc.vector.tensor_scalar(
                                junk[:, :],
                                p_ps[:, :] if f16 else p_ps[:, :].bitcast(F32),
                                scTn_sb[:, j : j + 1],
                                60000.0 if (w16 and f16) else 3.0e38,
                                OP.mult,
                                OP.min,
                                accum_out=mneg[:, :],
                            )
                        else:
                            s_neg = smallp.tile([128, 1], F32, name="s_neg", tag="s_neg")
                            nc.vector.scalar_tensor_tensor(
                                out=junk[:, :],
                                in0=p_ps[:, :] if f16 else p_ps[:, :].bitcast(F32),
                                scalar=-1.0,
                                in1=dbcast[:, :],
                                op0=OP.mult,
                                op1=OP.mult,
                                accum_out=s_neg[:, :],
                            )
                            junk2 = junkp.tile([128, H], F32, name="junk2", tag="junk")
                            nc.vector.tensor_scalar(
                                junk2[:, :],
                                p_ps[:, :] if f16 else p_ps[:, :].bitcast(F32),
                                s_neg[:, :],
                                3.0e38,
                                OP.mult,
                                OP.min,
                                accum_out=mneg[:, :],
                            )
                            junk = junk2

                        # e = exp(s*Et - max), z = sum_h e  (z >= 1)
                        e = ep.tile([128, H], BF16, name="e", tag="e")
                        z = smallp.tile([128, 1], F32, name="z", tag="z")
                        nc.scalar.activation(
                            e[:, :],
                            junk[:, :],
                            AF.Exp,
                            bias=mneg[:, :],
                            scale=-1.0,
                            accum_out=z[:, :],
                        )
                        pend[j] = (e, z)
                        if j in lazy_pieces:
                            emit_piece(*lazy_pieces.pop(j))
                    if step >= b_lag:
                        k = step - b_lag
                        e, z = pend.pop(k)
                        r = smallp.tile([128, 1], F32, name="r", tag="r")
                        nc.vector.reciprocal(r[:, :], z[:, :])
                        rl = smallp.tile([128, 1], BF16, name="rl", tag="rl")
                        if rl_engine == "vector":
                            nc.vector.tensor_scalar(rl[:, :], r[:, :], 1.0, None, OP.mult)
                        elif rl_engine == "gpsimd":
                            nc.gpsimd.tensor_scalar(rl[:, :], r[:, :], 1.0, None, OP.mult)
                        else:
                            nc.scalar.copy(rl[:, :], r[:, :])
                        # out[h] += sum_t r_t * e[t, h]
                        if o_cols:
                            # flipped: e-block stationary, rl streams 1 col ->
                            # out col [128, 1] per h-block (h on partitions).
                            # All 64 matmuls join the group opened by the
                            # zero write; the very last one closes it.
                            for i in range(NHC):
                                nc.tensor.matmul(
                                    o_ps[:, i : i + 1],
                                    lhsT=e[:, 128 * i : 128 * (i + 1)],
                                    rhs=rl[:, :],
                                    start=False,
                                    stop=(k == NTC - 1 and i == NHC - 1),
                                )
                        else:
                            for half in range(2):
                                if o_compact:
                                    o_slice = o_ps[32 * half : 32 * half + 1, :]
                                else:
                                    o_slice = o_ps[0:1, 512 * half : 512 * half + 512]
                                nc.tensor.matmul(
                                    o_slice,
                                    lhsT=rl[:, :],
                                    rhs=e[:, 512 * half : 512 * half + 512],
                                    start=(k == 0),
                                    stop=(k == NTC - 1),
                                )

                if o_defer and b < B_LOC - 1:
                    pending_flush = (b, o_ps)
                else:
                    flush_out(b, o_ps)

    if split_waits:
        _split_waits(nc)
    return nc


def make_in_maps(decoder_hidden, encoder_outputs, host_cast=True):
    dec = np.ascontiguousarray(np.asarray(decoder_hidden, dtype=np.float32))
    enc = np.asarray(encoder_outputs)
    if host_cast:
        enc = enc.astype(np.float16)  # rounding host-side; device loads are 2B/elem
    else:
        enc = enc.astype(np.float32, copy=False)
    assert dec.shape == (B, H) and enc.shape == (H, B, T)
    ident = np.eye(128, dtype=np.float32)
    in_maps = []
    for k in range(N_CORES):
        bsl = slice(k * B_LOC, (k + 1) * B_LOC)
        in_maps.append(
            {
                "enc": np.ascontiguousarray(enc[:, bsl, :]),
                "dec": np.ascontiguousarray(dec[bsl, :]),
                "ident": ident,
            }
        )
    return in_maps


_PROGRAM = None


def kernel(**inputs) -> np.ndarray:
    global _PROGRAM
    if _PROGRAM is None:
        _PROGRAM = build_program()
    in_maps = make_in_maps(inputs["decoder_hidden"], inputs["encoder_outputs"])
    res = run_bass_kernel_spmd(_PROGRAM, in_maps, core_ids=list(range(N_CORES)))
    return np.concatenate([r["out"] for r in res.results], axis=0)



# revision 45
# speedup vs baseline: 1.1473x; 1.1473x over previous
"""Trainium2 Bass kernel (v6) for nn_Attn: batched column-softmax attention energies.

Math (per batch element b):
    E = encoder_outputs[:, b, :]            # [H, T]
    d = decoder_hidden[b]                   # [H]
    s = E^T d                               # [T]  (scores)
    w[h, t] = E[h, t] * s[t]
    sm = softmax over h of w (per column t)
    out[b, h] = sum_t sm[h, t]

Design (per core, data parallel over batch: 8 cores x 8 batch elements):
    - v6: E is cast to float16 on the HOST in make_in_maps, so DRAM holds
      16MB/core instead of 32MB -- HBM read traffic (the real-HW binding
      resource under device contention) is HALVED, and the loads become
      plain HWDGE (sync) DMAs with 2KB-contiguous descriptors instead of
      SWDGE casting DMAs (~2us fixed cost each, Q7 descriptor generation).
      Numerically identical to the old device-side casting DMA (both RNE):
      HW rel err 1.0e-2 vs the 2e-2 gate (bf16 E fails at 3.8e-2).
    - E transposes as fp16 transpose-mode matmuls: 1.0 cyc/row (vs 2.0 fp32),
      each Et PSUM tile one bank (pp_bufs=3 deepens the pipeline).
    - scores on PE: sneg_row[1,T] = sum_i (-d_i)^T @ E_i per T-half, then tiny
      fp32 PE transposes give per-partition score columns scT[128, NTC].
      (score_lazy / nat_bufs=5 / last-b lag=1 were tried and are model-neutral;
      left OFF -- only model-positive, HW-A/B-validated changes ship.)
    - DVE: ONE fused pass per chunk: junk(fp16) = -s*Et, accum min -> mneg =
      -max_h(s*Et); fp16 PSUM in + fp16 out -> 2x_1p mode. Plus recip 1/z,
      bf16 cast, and s-score PSUM->SBUF copies.
    - ACT: e = exp(-junk + mneg) as bf16, accum z (z >= 1). v6: ACT does
      ONLY the 64 exps -- the old [1,512] output-row copies are gone.
    - v6 output path (o_cols): final matmuls are FLIPPED -- lhsT = e-block
      [128t,128h] (stationary), rhs = rl [128,1] -> out COLUMN [128,1] per
      h-block, accumulated over t-chunks into one PSUM bank [128, NHC].
      TRN2 allows one accumulation group per 2KB zero region, so the group
      is opened once per b by a full-region zero write (I^T @ 0, start=True);
      all 64 column matmuls join it (start=False) and the last one stops.
      Flush per b = tiny DVE copy [128,8] + one small PE transpose ->
      [8,128] + DVE copy + a DMA with 8x512B descriptors. This removes
      ~10us/core of ACT row-copies vs v5 (ACT busy 88.2 -> 78.4us) at the
      cost of ~2us of DVE/PE dust; flush is deferred into the next b's
      chunk loop (o_defer) so it never blocks the softmax-critical DVE chain.
    - back-half ops (recip/cast/final matmuls) are emitted 2 chunks late so
      the in-order PE/DVE queues never stall on the softmax chain.
    - pe_warm dummy transposes bridge the decT->scores PE idle gap (~3.3us,
      right at the HAM 3.4us re-throttle window); score_head_split emits a
      chunk-0-only b0 scores piece so the first exp fires ~1.5us earlier.
    - engine busy (TimelineSim cost model, per core): ACT 78.4us (critical,
      zero mid-gaps; 64 exps at (1024+446)cyc -- the fixed part is intrinsic
      instruction overhead), PE ~57us modeled (~82us real incl unmodeled
      LDWEIGHTS), DVE 58us, DMA 47us; predicted total 92.0us vs 105.9us for
      v5 (b0_sliver: b0's first 0.5MB lands alone so exp0 fires ~4us sooner).
      Remaining slack is the DMA-gated head (~9us) and the flush tail
      (~4us). Real-HW marginal timing is dominated by shared-device noise
      (210us..3.5ms for the SAME kernel minutes apart); best observed v6:
      210us vs v5 baseline 333us.
"""

import numpy as np

import concourse.bass as bass
import concourse.mybir as mybir
from concourse.bass_utils import run_bass_kernel_spmd
from concourse.tile import TileContext

H = 1024
B = 64
T = 1024
N_CORES = 8
B_LOC = B // N_CORES  # 8 batch elements per core
NHC = H // 128        # 8 h-chunks
NTC = T // 128        # 8 t-chunks

F32 = mybir.dt.float32
F32R = mybir.dt.float32r
F16 = mybir.dt.float16
BF16 = mybir.dt.bfloat16


def _split_waits(nc, max_waits=1):
    """Workaround for this container's walrus: instructions accept only one
    sync-wait; hoist extra waits onto single-wait Drain carriers."""
    n_new = 0
    for f in nc.m.functions:
        for blk in f.blocks:
            new_insts = []
            for inst in blk.instructions:
                si = inst.sync_info
                if si is not None and si.on_wait is not None and len(si.on_wait) > max_waits:
                    waits = list(si.on_wait)
                    while len(waits) > max_waits:
                        w = waits.pop(0)
                        d = mybir.InstDrain(
                            name=f"I-ws-{nc.next_id()}", ins=[], outs=[]
                        )
                        d.engine = inst.engine
                        d.sync_info = mybir.SyncInfo(on_wait=[w], on_update=[])
                        new_insts.append(d)
                        n_new += 1
                    si.on_wait = waits
                new_insts.append(inst)
            blk.instructions = new_insts
    return n_new


def build_program(
    host_cast=True,      # enc pre-cast to fp16 on HOST: halves HBM traffic, HWDGE loads
    f32r=True,           # f32r natt (casting DMA) + f32r transposes + f32r s-matmuls
    f16=True,            # fp16 natt (1 cyc/row transposes, 1-bank Et PSUM)
    score_pe=True,       # scores via PE (else DVE pass like v1)
    o_compact=True,      # out accum as [2,512] in one PSUM bank, double buffered
    split_nat_dma=2,     # natt loaded in this many DMAs (pipeline head start; 2 aligns with the score-row halves and halves SWDGE descriptor count vs 4)
    rl_engine="vector",
    lag=2,               # chunks of emission lag for recip/cast/final-matmul
    nat_bufs=3,
    pp_bufs=3,
    w16=True,            # fp16 junk (-s*Et scratch): 2-byte in+out => DVE 2x_1p mode
    s2_engine="vector",   # engine for s2 PSUM->SBUF copies: scalar|vector
    o_direct_dma=False,   # dead: bass forbids PSUM-source DMA (in_ must be SBUF/DRAM)
    orow_engine="scalar", # engine for o PSUM->SBUF copies: scalar|vector (vector mis-executes the partition-32 copy on real HW)|vector33 (ONE DVE copy spanning partitions 0..32, base 0)|scalar33
    nat_prefetch=False,  # issue b=0 natt DMAs before ident/dec (head start)
    o_defer=True,        # emit b's orow copy + out DMA inside b+1's chunk loop
    o_cols=True,         # out accum as PSUM COLUMNS [128, NHC] (lhsT=e-block,
                         # rhs=rl): flush = tiny DVE copy + small PE transpose
                         # + 8x512B DMA; removes the [1,512] ACT row copies
    score_lazy=False,    # b=0: emit scores piece 1 after chunk-2's exp
                         # (model-neutral; measured noisy-negative on HW -> off)
    pe_warm=18,          # dummy identsb transposes after decT so PE spans the
                         # HAM warmup window busy and the b0 scores run at 2.4GHz
    score_head_split=True,  # b=0: emit a chunk-0-only scores piece first
    b0_sliver=True,      # b=0 loads as 256+256+512 so chunk-0 waits on 0.5MB
    head_quarters=False, # b=0 quarter-granular head: helps pre-split-waits (-1.2us) but the wait-split drains land worse (+0.8us net) -> off
    junk_bufs=3,
    e_bufs=4,
    small_bufs=8,
    split_waits=True,
):
    nc = bass.Bass("TRN2", debug=False, num_devices=N_CORES)
    if host_cast:
        f16 = True
    enc_h = nc.dram_tensor(
        "enc", [H, B_LOC, T], F16 if host_cast else F32, kind="ExternalInput"
    )
    dec_h = nc.dram_tensor("dec", [B_LOC, H], F32, kind="ExternalInput")
    ident_h = nc.dram_tensor("ident", [128, 128], F32, kind="ExternalInput")
    out_h = nc.dram_tensor("out", [B_LOC, H], F32, kind="ExternalOutput")

    enc = enc_h.ap()
    dec = dec_h.ap()
    ident = ident_h.ap()
    out = out_h.ap()

    AF = mybir.ActivationFunctionType
    OP = mybir.AluOpType

    if f16:
        f32r = False
    NAT_DT = F16 if f16 else (F32R if f32r else F32)
    PP_DT = F16 if f16 else F32

    def mm_trans(out_ap, lhsT, rhs, **kw):
        if f32r:
            out_ap = out_ap.bitcast(F32R)
        nc.tensor.matmul(out_ap, lhsT=lhsT, rhs=rhs, is_transpose=True, **kw)

    with TileContext(nc) as tc:
        with (
            tc.tile_pool(name="const", bufs=1) as constp,
            tc.tile_pool(name="natp", bufs=nat_bufs) as natp,
            tc.tile_pool(name="junkp", bufs=junk_bufs) as junkp,
            tc.tile_pool(name="ep", bufs=e_bufs) as ep,
            tc.tile_pool(name="srowp", bufs=2) as srowp,
            tc.tile_pool(name="sctp", bufs=2) as sctp,
            tc.tile_pool(name="smallp", bufs=small_bufs) as smallp,
            tc.tile_pool(name="rowp", bufs=2) as rowp,
            tc.tile_pool(name="dbp", bufs=2) as dbp,  # only if not score_pe
            tc.tile_pool(name="ps_p", bufs=pp_bufs, space="PSUM") as ps_p,
            tc.tile_pool(name="ps_o", bufs=2 if (o_compact or o_cols) else 1, space="PSUM") as ps_o,
            tc.tile_pool(name="ps_s", bufs=2, space="PSUM") as ps_s,   # 2 banks
            tc.tile_pool(name="ps_x", bufs=1, space="PSUM") as ps_x,   # out transpose
        ):
            # issue b=0's natt loads FIRST so the big transfer heads the HWDGE
            # queue (ident/dec are tiny and their consumers run later anyway)
            natt_pre = None
            if host_cast and nat_prefetch:
                # head order: natt[b0] first half -> ident+dec (tiny) -> rest,
                # so scores piece 0 and decT are both ready ~as early as possible
                enc_b0 = enc[:, 0, :].rearrange("(ii p) t -> p ii t", p=128)
                natt_pre = natp.tile([128, NHC, T], NAT_DT, name="natt", tag="nat")
                tw0 = T // split_nat_dma
                nc.sync.dma_start(out=natt_pre[:, :, 0:tw0], in_=enc_b0[:, :, 0:tw0])

            # ident/dec stay on the sync ring by default; under nat_prefetch
            # they ride the idle SWDGE (gpsimd) dispatcher so the natt[b0]
            # transfer heads the sync HWDGE ring alone (ACT ring is unusable:
            # DMA dispatches there pollute the exp-critical ACT queue)
            aux_dma = nc.gpsimd if nat_prefetch else nc.sync
            identsb = constp.tile([128, 128], F32, name="identsb")
            aux_dma.dma_start(out=identsb[:, :], in_=ident)
            zero_sb = None
            if o_cols:
                zero_sb = constp.tile([128, NHC], F32, name="zero_sb")
                nc.vector.tensor_scalar(zero_sb[:, :], identsb[:, 0:NHC], 0.0, None, OP.mult)
            if f32r or f16:
                identr = constp.tile([128, 128], NAT_DT, name="identr")
                nc.vector.tensor_scalar(identr[:, :], identsb[:, :], 1.0, None, OP.mult)
            else:
                identr = identsb

            decTn_sb = None
            if score_pe:
                # dec natural [B_LOC, H] (one contiguous 32KB DMA)
                dec_nat = constp.tile([B_LOC, H], F32, name="dec_nat")
                aux_dma.dma_start(out=dec_nat[:, :], in_=dec)
                # decT[p, i, b] = d[b, 128i+p]; negate while copying to SBUF
                decT_ps = ps_s.tile([128, NHC, B_LOC], F32, name="decT_ps", tag="ps_s")
                for i in range(NHC):
                    nc.tensor.matmul(
                        decT_ps[:, i, :],
                        lhsT=dec_nat[:, 128 * i : 128 * (i + 1)],
                        rhs=identsb[0:B_LOC, 0:B_LOC],
                        is_transpose=True,
                    )
                decTn_sb = constp.tile([128, NHC, B_LOC], NAT_DT, name="decTn_sb")
                nc.vector.tensor_scalar(
                    decTn_sb[:, :, :], decT_ps[:, :, :], -1.0, None, OP.mult
                )

            if natt_pre is not None:
                tw0 = T // split_nat_dma
                enc_b0 = enc[:, 0, :].rearrange("(ii p) t -> p ii t", p=128)
                for q in range(1, split_nat_dma):
                    tsl = slice(q * tw0, (q + 1) * tw0)
                    nc.sync.dma_start(out=natt_pre[:, :, tsl], in_=enc_b0[:, :, tsl])

            if pe_warm:
                # keep the PE array busy from decT until natt[b0] lands so the
                # HAM clock gate stays open (cold matmuls run at 1.2 not 2.4GHz)
                warm_ps = ps_x.tile([128, 128], F32, name="warm_ps", tag="ps_x")
                for _ in range(pe_warm):
                    nc.tensor.matmul(
                        warm_ps[:, :], lhsT=identsb[:, :], rhs=identsb[:, :],
                        is_transpose=True,
                    )

            def flush_out(bprev, o_psprev):
                if o_cols:
                    # o_psprev [128, NHC]: col i holds out[b, 128i + p] at
                    # partition p. Copy out (tiny), transpose to [NHC, 128]
                    # so the DMA writes 8 contiguous 512B runs.
                    o_sb = rowp.tile([128, NHC], F32, name="o_sb", tag="orow")
                    nc.vector.tensor_scalar(o_sb[:, :], o_psprev[:, :], 1.0, None, OP.mult)
                    x_ps = ps_x.tile([NHC, 128], F32, name="x_ps", tag="ps_x")
                    nc.tensor.matmul(
                        x_ps[:, :], lhsT=o_sb[:, :], rhs=identsb[:, :],
                        is_transpose=True,
                    )
                    x_sb = rowp.tile([NHC, 128], F32, name="x_sb", tag="orow")
                    nc.vector.tensor_scalar(x_sb[:, :], x_ps[:, :], 1.0, None, OP.mult)
                    out_b = out[bprev : bprev + 1, :].rearrange(
                        "o (ii p) -> (o ii) p", p=128
                    )
                    nc.sync.dma_start(out=out_b, in_=x_sb[:, :])
                    return
                if o_compact and o_direct_dma:
                    # straight PSUM -> DRAM, no SBUF staging
                    nc.sync.dma_start(out=out[bprev : bprev + 1, 0:512], in_=o_psprev[0:1, :])
                    nc.sync.dma_start(out=out[bprev : bprev + 1, 512:1024], in_=o_psprev[32:33, :])
                elif o_compact:
                    orow = rowp.tile([33, 512], F32, name="orow", tag="orow")
                    if orow_engine in ("strided", "vstrided"):
                        # one copy for both output rows via a partition-strided
                        # AP (base partition 0, stride 32) into rows {0,32}
                        if orow_engine == "strided":
                            nc.scalar.copy(orow[0:33:32, :], o_psprev[0:33:32, :])
                        else:
                            nc.vector.tensor_scalar(
                                orow[0:33:32, :], o_psprev[0:33:32, :], 1.0, None, OP.mult
                            )
                    elif orow_engine == "mixed":
                        # partition-0 row on DVE (safe base-0 AP), row 32 on ACT
                        nc.vector.tensor_scalar(orow[0:1, :], o_psprev[0:1, :], 1.0, None, OP.mult)
                        nc.scalar.copy(orow[32:33, :], o_psprev[32:33, :])
                    elif orow_engine == "vector33":
                        nc.vector.tensor_scalar(orow[0:33, :], o_psprev[0:33, :], 1.0, None, OP.mult)
                    elif orow_engine == "scalar33":
                        nc.scalar.copy(orow[0:33, :], o_psprev[0:33, :])
                    elif orow_engine == "vector":
                        nc.vector.tensor_scalar(orow[0:1, :], o_psprev[0:1, :], 1.0, None, OP.mult)
                        nc.vector.tensor_scalar(orow[32:33, :], o_psprev[32:33, :], 1.0, None, OP.mult)
                    else:
                        nc.scalar.copy(orow[0:1, :], o_psprev[0:1, :])
                        nc.scalar.copy(orow[32:33, :], o_psprev[32:33, :])
                    out_b = out[bprev : bprev + 1, :].rearrange("o (p f) -> (o p) f", p=2)
                    nc.sync.dma_start(out=out_b, in_=orow[0:33:32, :])
                else:
                    orow = rowp.tile([1, H], F32, name="orow", tag="orow")
                    nc.scalar.copy(orow[:, :], o_psprev[0:1, :])
                    nc.sync.dma_start(out=out[bprev : bprev + 1, :], in_=orow[:, :])

            pending_flush = None
            for b in range(B_LOC):
                # ---- natural-layout load: natt[p, i, t] = E[128*i + p, t]
                # (f32r: gpsimd SWDGE casting DMA rounds fp32 -> f32r in flight)
                enc_b = enc[:, b, :].rearrange("(ii p) t -> p ii t", p=128)
                if b == 0 and natt_pre is not None:
                    natt = natt_pre
                else:
                    natt = natp.tile([128, NHC, T], NAT_DT, name="natt", tag="nat")
                    if b == 0 and b0_sliver and host_cast:
                        # first load sliced [0:256]+[256:512]+[512:1024]: the
                        # chunk-0 score/transpose chain only waits on the first
                        # 0.5MB (512B descriptors -- still at line-rate size)
                        slices = [(0, 256), (256, 256), (512, 512)]
                    else:
                        nsplit = 4 if (head_quarters and b == 0) else split_nat_dma
                        tw = T // nsplit
                        slices = [(q * tw, tw) for q in range(nsplit)]
                    for lo, w in slices:
                        tsl = slice(lo, lo + w)
                        if host_cast:
                            # enc already fp16 in DRAM: plain HWDGE load, no cast
                            nc.sync.dma_start(out=natt[:, :, tsl], in_=enc_b[:, :, tsl])
                        elif f32r or f16:
                            nc.gpsimd.dma_start(out=natt[:, :, tsl], in_=enc_b[:, :, tsl])
                        else:
                            nc.sync.dma_start(out=natt[:, :, tsl], in_=enc_b[:, :, tsl])

                emit_piece = None
                lazy_pieces = {}
                if score_pe:
                    # ---- sneg_row[1, T] = sum_i (-d_i)^T @ E_i, per T-half so
                    # chunks j<4 only wait on natt's first half (pipeline head)
                    # (kept fp32: the [1,128]->[128,1] transposes below violate
                    # walrus's fp32r ISA restrictions in f32r form)
                    srow_sb = srowp.tile([1, T], F32, name="srow_sb", tag="srow")
                    scTn_sb = sctp.tile([128, NTC], F32, name="scTn_sb", tag="sct")
                    def emit_piece(lo, pw, natt=natt, srow_sb=srow_sb,
                                   scTn_sb=scTn_sb, b=b):
                        jlo, jpp = lo // 128, pw // 128
                        tsl = slice(lo, lo + pw)
                        s2 = ps_s.tile([1, pw], F32, name="s2", tag="ps_s")
                        for i in range(NHC):
                            nc.tensor.matmul(
                                s2[0:1, :],
                                lhsT=decTn_sb[:, i, b : b + 1],
                                rhs=natt[:, i, tsl],
                                start=(i == 0),
                                stop=(i == NHC - 1),
                            )
                        if s2_engine == "vector":
                            nc.vector.tensor_scalar(srow_sb[0:1, tsl], s2[0:1, :], 1.0, None, OP.mult)
                        else:
                            nc.scalar.copy(srow_sb[0:1, tsl], s2[0:1, :])
                        # transpose this piece's scores to columns:
                        # scT[t_p, j] = sneg[128j + t_p]
                        scT_ps = ps_s.tile([128, jpp], F32, name="scT_ps", tag="ps_s")
                        for jj in range(jpp):
                            j = jlo + jj
                            nc.tensor.matmul(
                                scT_ps[:, jj : jj + 1],
                                lhsT=srow_sb[0:1, 128 * j : 128 * (j + 1)],
                                rhs=identsb[0:1, 0:1],
                                is_transpose=True,
                            )
                        nc.vector.tensor_scalar(
                            scTn_sb[:, jlo : jlo + jpp],
                            scT_ps[:, :], 1.0, None, OP.mult,
                        )

                    if score_head_split and b == 0:
                        # chunk-0 scores alone first: the exp-critical path is
                        # 8 narrow matmuls instead of a 512-wide piece
                        pieces = [(0, 128), (128, 384), (512, 512)]
                    else:
                        pieces = [(0, 512), (512, 512)]
                    for idx, (lo, pw) in enumerate(pieces):
                        if score_lazy and b == 0 and lo >= 512:
                            lazy_pieces[lo // 128 - 2] = (lo, pw)
                        else:
                            emit_piece(lo, pw)
                else:
                    dbcast = dbp.tile([128, H], F32, name="dbcast", tag="dbcast")
                    nc.sync.dma_start(
                        out=dbcast[:, :], in_=dec[b : b + 1, :].to_broadcast([128, H])
                    )

                # ---- per t-chunk: transpose, softmax, accumulate output
                if o_cols:
                    o_ps = ps_o.tile([128, NHC], F32, name="o_ps", tag="ps_o")
                    # one accumulation group for the whole [128, NHC] zero
                    # region: open it with a full-region zero write (I.T @ 0)
                    nc.tensor.matmul(
                        o_ps[:, :], lhsT=identsb[:, :], rhs=zero_sb[:, :],
                        start=True, stop=False,
                    )
                elif o_compact:
                    o_ps = ps_o.tile([128, 512], F32, name="o_ps", tag="ps_o")
                else:
                    o_ps = ps_o.tile([1, H], F32, name="o_ps", tag="ps_o")
                # Per chunk j: [transposes (PE), fused mult+min (DVE), exp (ACT)]
                # emitted at step j; [recip, bf16 cast (DVE), 2 final matmuls
                # (PE)] emitted `lag` chunks later so the in-order PE/DVE
                # queues never stall waiting for the softmax chain of the
                # current chunk.
                pend = {}
                b_lag = lag
                for step in range(NTC + b_lag):
                    if step == 1 and pending_flush is not None:
                        flush_out(*pending_flush)
                        pending_flush = None
                    if step < NTC:
                        j = step
                        p_ps = ps_p.tile([128, H], PP_DT, name="p_ps", tag="ps_p")
                        for i in range(NHC):
                            mm_trans(
                                p_ps[:, 128 * i : 128 * (i + 1)],
                                lhsT=natt[:, i, 128 * j : 128 * (j + 1)],
                                rhs=identr[:, :],
                                start=(i % 4 == 0),
                                stop=(i % 4 == 3),
                            )
                        junk = junkp.tile([128, H], F16 if (w16 and f16) else F32,
                                          name="junk", tag="junk")
                        mneg = smallp.tile([128, 1], F32, name="mneg", tag="mneg")
                        if score_pe:
                            nc.vector.tensor_scalar(
                                junk[:, :],
                                p_ps[:, :] if f16 else p_ps[:, :].bitcast(F32),
                                scTn_sb[:, j : j + 1],
                                60000.0 if (w16 and f16) else 3.0e38,
                                OP.mult,
                                OP.min,
                                accum_out=mneg[:, :],
                            )
                        else:
                            s_neg = smallp.tile([128, 1], F32, name="s_neg", tag="s_neg")
                            nc.vector.scalar_tensor_tensor(
                                out=junk[:, :],
                                in0=p_ps[:, :] if f16 else p_ps[:, :].bitcast(F32),
                                scalar=-1.0,
                                in1=dbcast[:, :],
                                op0=OP.mult,
                                op1=OP.mult,
                                accum_out=s_neg[:, :],
                            )
                            junk2 = junkp.tile([128, H], F32, name="junk2", tag="junk")
                            nc.vector.tensor_scalar(
                                junk2[:, :],
                                p_ps[:, :] if f16 else p_ps[:, :].bitcast(F32),
                                s_neg[:, :],
                                3.0e38,
                                OP.mult,
                                OP.min,
                                accum_out=mneg[:, :],
                            )
                            junk = junk2

                        # e = exp(s*Et - max), z = sum_h e  (z >= 1)
                        e = ep.tile([128, H], BF16, name="e", tag="e")
                        z = smallp.tile([128, 1], F32, name="z", tag="z")
                        nc.scalar.activation(
                            e[:, :],
                            junk[:, :],
                            AF.Exp,
                            bias=mneg[:, :],
                            scale=-1.0,
                            accum_out=z[:, :],
                        )
                        pend[j] = (e, z)
                        if j in lazy_pieces:
                            emit_piece(*lazy_pieces.pop(j))
                    if step >= b_lag:
                        k = step - b_lag
                        e, z = pend.pop(k)
                        r = smallp.tile([128, 1], F32, name="r", tag="r")
                        nc.vector.reciprocal(r[:, :], z[:, :])
                        rl = smallp.tile([128, 1], BF16, name="rl", tag="rl")
                        if rl_engine == "vector":
                            nc.vector.tensor_scalar(rl[:, :], r[:, :], 1.0, None, OP.mult)
                        elif rl_engine == "gpsimd":
                            nc.gpsimd.tensor_scalar(rl[:, :], r[:, :], 1.0, None, OP.mult)
                        else:
                            nc.scalar.copy(rl[:, :], r[:, :])
                        # out[h] += sum_t r_t * e[t, h]
                        if o_cols:
                            # flipped: e-block stationary, rl streams 1 col ->
                            # out col [128, 1] per h-block (h on partitions).
                            # All 64 matmuls join the group opened by the
                            # zero write; the very last one closes it.
                            for i in range(NHC):
                                nc.tensor.matmul(
                                    o_ps[:, i : i + 1],
                                    lhsT=e[:, 128 * i : 128 * (i + 1)],
                                    rhs=rl[:, :],
                                    start=False,
                                    stop=(k == NTC - 1 and i == NHC - 1),
                                )
                        else:
                            for half in range(2):
                                if o_compact:
                                    o_slice = o_ps[32 * half : 32 * half + 1, :]
                                else:
                                    o_slice = o_ps[0:1, 512 * half : 512 * half + 512]
                                nc.tensor.matmul(
                                    o_slice,
                                    lhsT=rl[:, :],
                                    rhs=e[:, 512 * half : 512 * half + 512],
                                    start=(k == 0),
                                    stop=(k == NTC - 1),
                                )

                if o_defer and b < B_LOC - 1:
                    pending_flush = (b, o_ps)
                else:
                    flush_out(b, o_ps)

    if split_waits:
        _split_waits(nc)
    return nc


def make_in_maps(decoder_hidden, encoder_outputs, host_cast=True):
    dec = np.ascontiguousarray(np.asarray(decoder_hidden, dtype=np.float32))
    enc = np.asarray(encoder_outputs)
    if host_cast:
        enc = enc.astype(np.float16)  # rounding host-side; device loads are 2B/elem
    else:
        enc = enc.astype(np.float32, copy=False)
    assert dec.shape == (B, H) and enc.shape == (H, B, T)
    ident = np.eye(128, dtype=np.float32)
    in_maps = []
    for k in range(N_CORES):
        bsl = slice(k * B_LOC, (k + 1) * B_LOC)
        in_maps.append(
            {
                "enc": np.ascontiguousarray(enc[:, bsl, :]),
                "dec": np.ascontiguousarray(dec[bsl, :]),
                "ident": ident,
            }
        )
    return in_maps


_PROGRAM = None


def kernel(**inputs) -> np.ndarray:
    global _PROGRAM
    if _PROGRAM is None:
        _PROGRAM = build_program()
    in_maps = make_in_maps(inputs["decoder_hidden"], inputs["encoder_outputs"])
    res = run_bass_kernel_spmd(_PROGRAM, in_maps, core_ids=list(range(N_CORES)))
    return np.concatenate([r["out"] for r in res.results], axis=0)



# revision 46
# speedup vs baseline: 1.2544x; 1.0933x over previous
"""Trainium2 Bass kernel (v6) for nn_Attn: batched column-softmax attention energies.

Math (per batch element b):
    E = encoder_outputs[:, b, :]            # [H, T]
    d = decoder_hidden[b]                   # [H]
    s = E^T d                               # [T]  (scores)
    w[h, t] = E[h, t] * s[t]
    sm = softmax over h of w (per column t)
    out[b, h] = sum_t sm[h, t]

Design (per core, data parallel over batch: 8 cores x 8 batch elements):
    - v6: E is cast to float16 on the HOST in make_in_maps, so DRAM holds
      16MB/core instead of 32MB -- HBM read traffic (the real-HW binding
      resource under device contention) is HALVED, and the loads become
      plain HWDGE (sync) DMAs with 2KB-contiguous descriptors instead of
      SWDGE casting DMAs (~2us fixed cost each, Q7 descriptor generation).
      Numerically identical to the old device-side casting DMA (both RNE):
      HW rel err 1.0e-2 vs the 2e-2 gate (bf16 E fails at 3.8e-2).
    - E transposes as fp16 transpose-mode matmuls: 1.0 cyc/row (vs 2.0 fp32),
      each Et PSUM tile one bank (pp_bufs=3 deepens the pipeline).
    - scores on PE: sneg_row[1,T] = sum_i (-d_i)^T @ E_i per T-half, then tiny
      fp32 PE transposes give per-partition score columns scT[128, NTC].
      (score_lazy / nat_bufs=5 / last-b lag=1 were tried and are model-neutral;
      left OFF -- only model-positive, HW-A/B-validated changes ship.)
    - DVE: ONE fused pass per chunk: junk(fp16) = -s*Et, accum min -> mneg =
      -max_h(s*Et); fp16 PSUM in + fp16 out -> 2x_1p mode. Plus recip 1/z,
      bf16 cast, and s-score PSUM->SBUF copies.
    - ACT: e = exp(-junk + mneg) as bf16, accum z (z >= 1). v6: ACT does
      ONLY the 64 exps -- the old [1,512] output-row copies are gone.
    - v6 output path (o_cols): final matmuls are FLIPPED -- lhsT = e-block
      [128t,128h] (stationary), rhs = rl [128,1] -> out COLUMN [128,1] per
      h-block, accumulated over t-chunks into one PSUM bank [128, NHC].
      TRN2 allows one accumulation group per 2KB zero region, so the group
      is opened once per b by a full-region zero write (I^T @ 0, start=True);
      all 64 column matmuls join it (start=False) and the last one stops.
      Flush per b = tiny DVE copy [128,8] + one small PE transpose ->
      [8,128] + DVE copy + a DMA with 8x512B descriptors. This removes
      ~10us/core of ACT row-copies vs v5 (ACT busy 88.2 -> 78.4us) at the
      cost of ~2us of DVE/PE dust; flush is deferred into the next b's
      chunk loop (o_defer) so it never blocks the softmax-critical DVE chain.
    - back-half ops (recip/cast/final matmuls) are emitted 2 chunks late so
      the in-order PE/DVE queues never stall on the softmax chain.
    - pe_warm dummy transposes bridge the decT->scores PE idle gap (~3.3us,
      right at the HAM 3.4us re-throttle window); score_head_split emits a
      chunk-0-only b0 scores piece so the first exp fires ~1.5us earlier.
    - engine busy (TimelineSim cost model, per core): ACT 78.4us (critical,
      zero mid-gaps; 64 exps at (1024+446)cyc -- the fixed part is intrinsic
      instruction overhead), PE ~57us modeled (~82us real incl unmodeled
      LDWEIGHTS), DVE 58us, DMA 47us; predicted total 92.0us vs 105.9us for
      v5 (b0_sliver: b0's first 0.5MB lands alone so exp0 fires ~4us sooner).
      Remaining slack is the DMA-gated head (~9us) and the flush tail
      (~4us). Real-HW marginal timing is dominated by shared-device noise
      (210us..3.5ms for the SAME kernel minutes apart); best observed v6:
      210us vs v5 baseline 333us.
"""

import numpy as np

import concourse.bass as bass
import concourse.mybir as mybir
from concourse.bass_utils import run_bass_kernel_spmd
from concourse.tile import TileContext

H = 1024
B = 64
T = 1024
N_CORES = 8
B_LOC = B // N_CORES  # 8 batch elements per core
NHC = H // 128        # 8 h-chunks
NTC = T // 128        # 8 t-chunks

F32 = mybir.dt.float32
F32R = mybir.dt.float32r
F16 = mybir.dt.float16
BF16 = mybir.dt.bfloat16


def _split_waits(nc, max_waits=1):
    """Workaround for this container's walrus: instructions accept only one
    sync-wait; hoist extra waits onto single-wait Drain carriers."""
    n_new = 0
    for f in nc.m.functions:
        for blk in f.blocks:
            new_insts = []
            for inst in blk.instructions:
                si = inst.sync_info
                if si is not None and si.on_wait is not None and len(si.on_wait) > max_waits:
                    waits = list(si.on_wait)
                    while len(waits) > max_waits:
                        w = waits.pop(0)
                        d = mybir.InstDrain(
                            name=f"I-ws-{nc.next_id()}", ins=[], outs=[]
                        )
                        d.engine = inst.engine
                        d.sync_info = mybir.SyncInfo(on_wait=[w], on_update=[])
                        new_insts.append(d)
                        n_new += 1
                    si.on_wait = waits
                new_insts.append(inst)
            blk.instructions = new_insts
    return n_new


def build_program(
    host_cast=True,      # enc pre-cast to fp16 on HOST: halves HBM traffic, HWDGE loads
    f32r=True,           # f32r natt (casting DMA) + f32r transposes + f32r s-matmuls
    f16=True,            # fp16 natt (1 cyc/row transposes, 1-bank Et PSUM)
    score_pe=True,       # scores via PE (else DVE pass like v1)
    o_compact=True,      # out accum as [2,512] in one PSUM bank, double buffered
    split_nat_dma=2,     # natt loaded in this many DMAs (pipeline head start; 2 aligns with the score-row halves and halves SWDGE descriptor count vs 4)
    rl_engine="vector",
    lag=2,               # chunks of emission lag for recip/cast/final-matmul
    nat_bufs=3,
    pp_bufs=3,
    w16=True,            # fp16 junk (-s*Et scratch): 2-byte in+out => DVE 2x_1p mode
    s2_engine="vector",   # engine for s2 PSUM->SBUF copies: scalar|vector
    o_direct_dma=False,   # dead: bass forbids PSUM-source DMA (in_ must be SBUF/DRAM)
    orow_engine="scalar", # engine for o PSUM->SBUF copies: scalar|vector (vector mis-executes the partition-32 copy on real HW)|vector33 (ONE DVE copy spanning partitions 0..32, base 0)|scalar33
    nat_prefetch=False,  # issue b=0 natt DMAs before ident/dec (head start)
    o_defer=True,        # emit b's orow copy + out DMA inside b+1's chunk loop
    o_cols=True,         # out accum as PSUM COLUMNS [128, NHC] (lhsT=e-block,
                         # rhs=rl): flush = tiny DVE copy + small PE transpose
                         # + 8x512B DMA; removes the [1,512] ACT row copies
    score_lazy=False,    # b=0: emit scores piece 1 after chunk-2's exp
                         # (model-neutral; measured noisy-negative on HW -> off)
    pe_warm=8,           # dummy identsb transposes after decT bridging PE idle
                         # before b0 scores (HAM re-throttle guard; with b0_sliver
                         # the gap is small, so 8 bounds queue-blocking at 0.9us)
    score_head_split=True,  # b=0: emit a chunk-0-only scores piece first
    b0_sliver=True,      # b=0 loads as 256+256+512 so chunk-0 waits on 0.5MB
    head_quarters=False, # b=0 quarter-granular head: helps pre-split-waits (-1.2us) but the wait-split drains land worse (+0.8us net) -> off
    junk_bufs=3,
    e_bufs=4,
    small_bufs=8,
    split_waits=True,
):
    nc = bass.Bass("TRN2", debug=False, num_devices=N_CORES)
    if host_cast:
        f16 = True
    enc_h = nc.dram_tensor(
        "enc", [H, B_LOC, T], F16 if host_cast else F32, kind="ExternalInput"
    )
    dec_h = nc.dram_tensor("dec", [B_LOC, H], F32, kind="ExternalInput")
    ident_h = nc.dram_tensor("ident", [128, 128], F32, kind="ExternalInput")
    out_h = nc.dram_tensor("out", [B_LOC, H], F32, kind="ExternalOutput")

    enc = enc_h.ap()
    dec = dec_h.ap()
    ident = ident_h.ap()
    out = out_h.ap()

    AF = mybir.ActivationFunctionType
    OP = mybir.AluOpType

    if f16:
        f32r = False
    NAT_DT = F16 if f16 else (F32R if f32r else F32)
    PP_DT = F16 if f16 else F32

    def mm_trans(out_ap, lhsT, rhs, **kw):
        if f32r:
            out_ap = out_ap.bitcast(F32R)
        nc.tensor.matmul(out_ap, lhsT=lhsT, rhs=rhs, is_transpose=True, **kw)

    with TileContext(nc) as tc:
        with (
            tc.tile_pool(name="const", bufs=1) as constp,
            tc.tile_pool(name="natp", bufs=nat_bufs) as natp,
            tc.tile_pool(name="junkp", bufs=junk_bufs) as junkp,
            tc.tile_pool(name="ep", bufs=e_bufs) as ep,
            tc.tile_pool(name="srowp", bufs=2) as srowp,
            tc.tile_pool(name="sctp", bufs=2) as sctp,
            tc.tile_pool(name="smallp", bufs=small_bufs) as smallp,
            tc.tile_pool(name="rowp", bufs=2) as rowp,
            tc.tile_pool(name="dbp", bufs=2) as dbp,  # only if not score_pe
            tc.tile_pool(name="ps_p", bufs=pp_bufs, space="PSUM") as ps_p,
            tc.tile_pool(name="ps_o", bufs=2 if (o_compact or o_cols) else 1, space="PSUM") as ps_o,
            tc.tile_pool(name="ps_s", bufs=2, space="PSUM") as ps_s,   # 2 banks
            tc.tile_pool(name="ps_x", bufs=1, space="PSUM") as ps_x,   # out transpose
        ):
            # issue b=0's natt loads FIRST so the big transfer heads the HWDGE
            # queue (ident/dec are tiny and their consumers run later anyway)
            natt_pre = None
            if host_cast and nat_prefetch:
                # head order: natt[b0] first half -> ident+dec (tiny) -> rest,
                # so scores piece 0 and decT are both ready ~as early as possible
                enc_b0 = enc[:, 0, :].rearrange("(ii p) t -> p ii t", p=128)
                natt_pre = natp.tile([128, NHC, T], NAT_DT, name="natt", tag="nat")
                tw0 = T // split_nat_dma
                nc.sync.dma_start(out=natt_pre[:, :, 0:tw0], in_=enc_b0[:, :, 0:tw0])

            # ident/dec stay on the sync ring by default; under nat_prefetch
            # they ride the idle SWDGE (gpsimd) dispatcher so the natt[b0]
            # transfer heads the sync HWDGE ring alone (ACT ring is unusable:
            # DMA dispatches there pollute the exp-critical ACT queue)
            aux_dma = nc.gpsimd if nat_prefetch else nc.sync
            identsb = constp.tile([128, 128], F32, name="identsb")
            aux_dma.dma_start(out=identsb[:, :], in_=ident)
            zero_sb = None
            if o_cols:
                zero_sb = constp.tile([128, NHC], F32, name="zero_sb")
                nc.vector.tensor_scalar(zero_sb[:, :], identsb[:, 0:NHC], 0.0, None, OP.mult)
            if f32r or f16:
                identr = constp.tile([128, 128], NAT_DT, name="identr")
                nc.vector.tensor_scalar(identr[:, :], identsb[:, :], 1.0, None, OP.mult)
            else:
                identr = identsb

            decTn_sb = None
            if score_pe:
                # dec natural [B_LOC, H] (one contiguous 32KB DMA)
                dec_nat = constp.tile([B_LOC, H], F32, name="dec_nat")
                aux_dma.dma_start(out=dec_nat[:, :], in_=dec)
                # decT[p, i, b] = d[b, 128i+p]; negate while copying to SBUF
                decT_ps = ps_s.tile([128, NHC, B_LOC], F32, name="decT_ps", tag="ps_s")
                for i in range(NHC):
                    nc.tensor.matmul(
                        decT_ps[:, i, :],
                        lhsT=dec_nat[:, 128 * i : 128 * (i + 1)],
                        rhs=identsb[0:B_LOC, 0:B_LOC],
                        is_transpose=True,
                    )
                decTn_sb = constp.tile([128, NHC, B_LOC], NAT_DT, name="decTn_sb")
                nc.vector.tensor_scalar(
                    decTn_sb[:, :, :], decT_ps[:, :, :], -1.0, None, OP.mult
                )

            if natt_pre is not None:
                tw0 = T // split_nat_dma
                enc_b0 = enc[:, 0, :].rearrange("(ii p) t -> p ii t", p=128)
                for q in range(1, split_nat_dma):
                    tsl = slice(q * tw0, (q + 1) * tw0)
                    nc.sync.dma_start(out=natt_pre[:, :, tsl], in_=enc_b0[:, :, tsl])

            if pe_warm:
                # keep the PE array busy from decT until natt[b0] lands so the
                # HAM clock gate stays open (cold matmuls run at 1.2 not 2.4GHz)
                warm_ps = ps_x.tile([128, 128], F32, name="warm_ps", tag="ps_x")
                for _ in range(pe_warm):
                    nc.tensor.matmul(
                        warm_ps[:, :], lhsT=identsb[:, :], rhs=identsb[:, :],
                        is_transpose=True,
                    )

            def flush_out(bprev, o_psprev):
                if o_cols:
                    # o_psprev [128, NHC]: col i holds out[b, 128i + p] at
                    # partition p. Copy out (tiny), transpose to [NHC, 128]
                    # so the DMA writes 8 contiguous 512B runs.
                    o_sb = rowp.tile([128, NHC], F32, name="o_sb", tag="orow")
                    nc.vector.tensor_scalar(o_sb[:, :], o_psprev[:, :], 1.0, None, OP.mult)
                    x_ps = ps_x.tile([NHC, 128], F32, name="x_ps", tag="ps_x")
                    nc.tensor.matmul(
                        x_ps[:, :], lhsT=o_sb[:, :], rhs=identsb[:, :],
                        is_transpose=True,
                    )
                    x_sb = rowp.tile([NHC, 128], F32, name="x_sb", tag="orow")
                    nc.vector.tensor_scalar(x_sb[:, :], x_ps[:, :], 1.0, None, OP.mult)
                    out_b = out[bprev : bprev + 1, :].rearrange(
                        "o (ii p) -> (o ii) p", p=128
                    )
                    nc.sync.dma_start(out=out_b, in_=x_sb[:, :])
                    return
                if o_compact and o_direct_dma:
                    # straight PSUM -> DRAM, no SBUF staging
                    nc.sync.dma_start(out=out[bprev : bprev + 1, 0:512], in_=o_psprev[0:1, :])
                    nc.sync.dma_start(out=out[bprev : bprev + 1, 512:1024], in_=o_psprev[32:33, :])
                elif o_compact:
                    orow = rowp.tile([33, 512], F32, name="orow", tag="orow")
                    if orow_engine in ("strided", "vstrided"):
                        # one copy for both output rows via a partition-strided
                        # AP (base partition 0, stride 32) into rows {0,32}
                        if orow_engine == "strided":
                            nc.scalar.copy(orow[0:33:32, :], o_psprev[0:33:32, :])
                        else:
                            nc.vector.tensor_scalar(
                                orow[0:33:32, :], o_psprev[0:33:32, :], 1.0, None, OP.mult
                            )
                    elif orow_engine == "mixed":
                        # partition-0 row on DVE (safe base-0 AP), row 32 on ACT
                        nc.vector.tensor_scalar(orow[0:1, :], o_psprev[0:1, :], 1.0, None, OP.mult)
                        nc.scalar.copy(orow[32:33, :], o_psprev[32:33, :])
                    elif orow_engine == "vector33":
                        nc.vector.tensor_scalar(orow[0:33, :], o_psprev[0:33, :], 1.0, None, OP.mult)
                    elif orow_engine == "scalar33":
                        nc.scalar.copy(orow[0:33, :], o_psprev[0:33, :])
                    elif orow_engine == "vector":
                        nc.vector.tensor_scalar(orow[0:1, :], o_psprev[0:1, :], 1.0, None, OP.mult)
                        nc.vector.tensor_scalar(orow[32:33, :], o_psprev[32:33, :], 1.0, None, OP.mult)
                    else:
                        nc.scalar.copy(orow[0:1, :], o_psprev[0:1, :])
                        nc.scalar.copy(orow[32:33, :], o_psprev[32:33, :])
                    out_b = out[bprev : bprev + 1, :].rearrange("o (p f) -> (o p) f", p=2)
                    nc.sync.dma_start(out=out_b, in_=orow[0:33:32, :])
                else:
                    orow = rowp.tile([1, H], F32, name="orow", tag="orow")
                    nc.scalar.copy(orow[:, :], o_psprev[0:1, :])
                    nc.sync.dma_start(out=out[bprev : bprev + 1, :], in_=orow[:, :])

            pending_flush = None
            for b in range(B_LOC):
                # ---- natural-layout load: natt[p, i, t] = E[128*i + p, t]
                # (f32r: gpsimd SWDGE casting DMA rounds fp32 -> f32r in flight)
                enc_b = enc[:, b, :].rearrange("(ii p) t -> p ii t", p=128)
                if b == 0 and natt_pre is not None:
                    natt = natt_pre
                else:
                    natt = natp.tile([128, NHC, T], NAT_DT, name="natt", tag="nat")
                    if b == 0 and b0_sliver and host_cast:
                        # first load sliced [0:256]+[256:512]+[512:1024]: the
                        # chunk-0 score/transpose chain only waits on the first
                        # 0.5MB (512B descriptors -- still at line-rate size)
                        slices = [(0, 256), (256, 256), (512, 512)]
                    else:
                        nsplit = 4 if (head_quarters and b == 0) else split_nat_dma
                        tw = T // nsplit
                        slices = [(q * tw, tw) for q in range(nsplit)]
                    for lo, w in slices:
                        tsl = slice(lo, lo + w)
                        if host_cast:
                            # enc already fp16 in DRAM: plain HWDGE load, no cast
                            nc.sync.dma_start(out=natt[:, :, tsl], in_=enc_b[:, :, tsl])
                        elif f32r or f16:
                            nc.gpsimd.dma_start(out=natt[:, :, tsl], in_=enc_b[:, :, tsl])
                        else:
                            nc.sync.dma_start(out=natt[:, :, tsl], in_=enc_b[:, :, tsl])

                emit_piece = None
                lazy_pieces = {}
                if score_pe:
                    # ---- sneg_row[1, T] = sum_i (-d_i)^T @ E_i, per T-half so
                    # chunks j<4 only wait on natt's first half (pipeline head)
                    # (kept fp32: the [1,128]->[128,1] transposes below violate
                    # walrus's fp32r ISA restrictions in f32r form)
                    srow_sb = srowp.tile([1, T], F32, name="srow_sb", tag="srow")
                    scTn_sb = sctp.tile([128, NTC], F32, name="scTn_sb", tag="sct")
                    def emit_piece(lo, pw, natt=natt, srow_sb=srow_sb,
                                   scTn_sb=scTn_sb, b=b):
                        jlo, jpp = lo // 128, pw // 128
                        tsl = slice(lo, lo + pw)
                        s2 = ps_s.tile([1, pw], F32, name="s2", tag="ps_s")
                        for i in range(NHC):
                            nc.tensor.matmul(
                                s2[0:1, :],
                                lhsT=decTn_sb[:, i, b : b + 1],
                                rhs=natt[:, i, tsl],
                                start=(i == 0),
                                stop=(i == NHC - 1),
                            )
                        if s2_engine == "vector":
                            nc.vector.tensor_scalar(srow_sb[0:1, tsl], s2[0:1, :], 1.0, None, OP.mult)
                        else:
                            nc.scalar.copy(srow_sb[0:1, tsl], s2[0:1, :])
                        # transpose this piece's scores to columns:
                        # scT[t_p, j] = sneg[128j + t_p]
                        scT_ps = ps_s.tile([128, jpp], F32, name="scT_ps", tag="ps_s")
                        for jj in range(jpp):
                            j = jlo + jj
                            nc.tensor.matmul(
                                scT_ps[:, jj : jj + 1],
                                lhsT=srow_sb[0:1, 128 * j : 128 * (j + 1)],
                                rhs=identsb[0:1, 0:1],
                                is_transpose=True,
                            )
                        nc.vector.tensor_scalar(
                            scTn_sb[:, jlo : jlo + jpp],
                            scT_ps[:, :], 1.0, None, OP.mult,
                        )

                    if score_head_split and b == 0:
                        # chunk-0 scores alone first: the exp-critical path is
                        # 8 narrow matmuls instead of a 512-wide piece
                        pieces = [(0, 128), (128, 384), (512, 512)]
                    else:
                        pieces = [(0, 512), (512, 512)]
                    for idx, (lo, pw) in enumerate(pieces):
                        if score_lazy and b == 0 and lo >= 512:
                            lazy_pieces[lo // 128 - 2] = (lo, pw)
                        else:
                            emit_piece(lo, pw)
                else:
                    dbcast = dbp.tile([128, H], F32, name="dbcast", tag="dbcast")
                    nc.sync.dma_start(
                        out=dbcast[:, :], in_=dec[b : b + 1, :].to_broadcast([128, H])
                    )

                # ---- per t-chunk: transpose, softmax, accumulate output
                if o_cols:
                    o_ps = ps_o.tile([128, NHC], F32, name="o_ps", tag="ps_o")
                    # one accumulation group for the whole [128, NHC] zero
                    # region: open it with a full-region zero write (I.T @ 0)
                    nc.tensor.matmul(
                        o_ps[:, :], lhsT=identsb[:, :], rhs=zero_sb[:, :],
                        start=True, stop=False,
                    )
                elif o_compact:
                    o_ps = ps_o.tile([128, 512], F32, name="o_ps", tag="ps_o")
                else:
                    o_ps = ps_o.tile([1, H], F32, name="o_ps", tag="ps_o")
                # Per chunk j: [transposes (PE), fused mult+min (DVE), exp (ACT)]
                # emitted at step j; [recip, bf16 cast (DVE), 2 final matmuls
                # (PE)] emitted `lag` chunks later so the in-order PE/DVE
                # queues never stall waiting for the softmax chain of the
                # current chunk.
                pend = {}
                b_lag = lag
                for step in range(NTC + b_lag):
                    if step == 1 and pending_flush is not None:
                        flush_out(*pending_flush)
                        pending_flush = None
                    if step < NTC:
                        j = step
                        p_ps = ps_p.tile([128, H], PP_DT, name="p_ps", tag="ps_p")
                        for i in range(NHC):
                            mm_trans(
                                p_ps[:, 128 * i : 128 * (i + 1)],
                                lhsT=natt[:, i, 128 * j : 128 * (j + 1)],
                                rhs=identr[:, :],
                                start=(i % 4 == 0),
                                stop=(i % 4 == 3),
                            )
                        junk = junkp.tile([128, H], F16 if (w16 and f16) else F32,
                                          name="junk", tag="junk")
                        mneg = smallp.tile([128, 1], F32, name="mneg", tag="mneg")
                        if score_pe:
                            nc.vector.tensor_scalar(
                                junk[:, :],
                                p_ps[:, :] if f16 else p_ps[:, :].bitcast(F32),
                                scTn_sb[:, j : j + 1],
                                60000.0 if (w16 and f16) else 3.0e38,
                                OP.mult,
                                OP.min,
                                accum_out=mneg[:, :],
                            )
                        else:
                            s_neg = smallp.tile([128, 1], F32, name="s_neg", tag="s_neg")
                            nc.vector.scalar_tensor_tensor(
                                out=junk[:, :],
                                in0=p_ps[:, :] if f16 else p_ps[:, :].bitcast(F32),
                                scalar=-1.0,
                                in1=dbcast[:, :],
                                op0=OP.mult,
                                op1=OP.mult,
                                accum_out=s_neg[:, :],
                            )
                            junk2 = junkp.tile([128, H], F32, name="junk2", tag="junk")
                            nc.vector.tensor_scalar(
                                junk2[:, :],
                                p_ps[:, :] if f16 else p_ps[:, :].bitcast(F32),
                                s_neg[:, :],
                                3.0e38,
                                OP.mult,
                                OP.min,
                                accum_out=mneg[:, :],
                            )
                            junk = junk2

                        # e = exp(s*Et - max), z = sum_h e  (z >= 1)
                        e = ep.tile([128, H], BF16, name="e", tag="e")
                        z = smallp.tile([128, 1], F32, name="z", tag="z")
                        nc.scalar.activation(
                            e[:, :],
                            junk[:, :],
                            AF.Exp,
                            bias=mneg[:, :],
                            scale=-1.0,
                            accum_out=z[:, :],
                        )
                        pend[j] = (e, z)
                        if j in lazy_pieces:
                            emit_piece(*lazy_pieces.pop(j))
                    if step >= b_lag:
                        k = step - b_lag
                        e, z = pend.pop(k)
                        r = smallp.tile([128, 1], F32, name="r", tag="r")
                        nc.vector.reciprocal(r[:, :], z[:, :])
                        rl = smallp.tile([128, 1], BF16, name="rl", tag="rl")
                        if rl_engine == "vector":
                            nc.vector.tensor_scalar(rl[:, :], r[:, :], 1.0, None, OP.mult)
                        elif rl_engine == "gpsimd":
                            nc.gpsimd.tensor_scalar(rl[:, :], r[:, :], 1.0, None, OP.mult)
                        else:
                            nc.scalar.copy(rl[:, :], r[:, :])
                        # out[h] += sum_t r_t * e[t, h]
                        if o_cols:
                            # flipped: e-block stationary, rl streams 1 col ->
                            # out col [128, 1] per h-block (h on partitions).
                            # All 64 matmuls join the group opened by the
                            # zero write; the very last one closes it.
                            for i in range(NHC):
                                nc.tensor.matmul(
                                    o_ps[:, i : i + 1],
                                    lhsT=e[:, 128 * i : 128 * (i + 1)],
                                    rhs=rl[:, :],
                                    start=False,
                                    stop=(k == NTC - 1 and i == NHC - 1),
                                )
                        else:
                            for half in range(2):
                                if o_compact:
                                    o_slice = o_ps[32 * half : 32 * half + 1, :]
                                else:
                                    o_slice = o_ps[0:1, 512 * half : 512 * half + 512]
                                nc.tensor.matmul(
                                    o_slice,
                                    lhsT=rl[:, :],
                                    rhs=e[:, 512 * half : 512 * half + 512],
                                    start=(k == 0),
                                    stop=(k == NTC - 1),
                                )

                if o_defer and b < B_LOC - 1:
                    pending_flush = (b, o_ps)
                else:
                    flush_out(b, o_ps)

    if split_waits:
        _split_waits(nc)
    return nc


def make_in_maps(decoder_hidden, encoder_outputs, host_cast=True):
    dec = np.ascontiguousarray(np.asarray(decoder_hidden, dtype=np.float32))
    enc = np.asarray(encoder_outputs)
    if host_cast:
        enc = enc.astype(np.float16)  # rounding host-side; device loads are 2B/elem
    else:
        enc = enc.astype(np.float32, copy=False)
    assert dec.shape == (B, H) and enc.shape == (H, B, T)
    ident = np.eye(128, dtype=np.float32)
    in_maps = []
    for k in range(N_CORES):
        bsl = slice(k * B_LOC, (k + 1) * B_LOC)
        in_maps.append(
            {
                "enc": np.ascontiguousarray(enc[:, bsl, :]),
                "dec": np.ascontiguousarray(dec[bsl, :]),
                "ident": ident,
            }
        )
    return in_maps


_PROGRAM = None


def kernel(**inputs) -> np.ndarray:
    global _PROGRAM
    if _PROGRAM is None:
        _PROGRAM = build_program()
    in_maps = make_in_maps(inputs["decoder_hidden"], inputs["encoder_outputs"])
    res = run_bass_kernel_spmd(_PROGRAM, in_maps, core_ids=list(range(N_CORES)))
    return np.concatenate([r["out"] for r in res.results], axis=0)

